# revision 35
# baseline (speedup 1.0000x reference)
"""Differential attention kernel for 8 Trainium2 NeuronCores — v8.

- fp8e4 DoubleRow projections, both-side residual compensated (0.75x bf16),
  weights pre-scaled x64 (raw W sits in e4m3's subnormal range).
- combined-probability PV: p = e1 - c*e2 with c = lam*s1/s2 per query,
  computed once per q-chunk -> ONE bf16 PV matmul pass (half the PV cost).
  Row sums via N=1 ones-matmuls (q-partitioned), PE-transposed to rows via
  a host identity, lam folded into c, x64 v-scale folded into the ones vec.
- filler-queue emission: chunk 4-7 K/V projections and the previous
  q-chunk's combine+PV drain into the ACT(exp)-bound score stream so the
  PE never idles while the scalar engine works through ~133us of exp.
- single 8-bank PSUM budget: score/proj share one [128,1024] ring (4),
  sums (1) + transpose rows (1) + packed PV accumulators (2).
"""

import math
import os
import time
from collections import deque
from contextlib import ExitStack

import ml_dtypes
import numpy as np

import concourse.bass as bass
from concourse import bacc
import concourse.mybir as mybir
import concourse.tile as tile
from concourse.bass_utils import run_bass_kernel_spmd

B, S, D = 4, 4096, 2048
HD = 128
DV = 256
SQ = S // 2
N_CORES = 8
DEPTH = 12
WSCALE = 64.0     # weight pre-scale before fp8 quantization
SCALE = (HD ** -0.5) / (WSCALE * WSCALE)   # scores carry WSCALE^2

DT_P = D // 128   # 16 d-tiles
DP = DT_P // 2    # 8 d-tile pairs
SKT = S // 128    # 32 key tiles
SKP = SKT // 2    # 16 key tile pairs
SC = S // 512     # 8 s-chunks
QC = SQ // 512    # 4 q-chunks
SQT = SQ // 128   # 16 q tiles

BF16 = mybir.dt.bfloat16
F32 = mybir.dt.float32
F8 = mybir.dt.float8e4
FP8NP = ml_dtypes.float8_e4m3fn

INPUT_NAMES = ("x8", "xd8", "wq", "wqd", "wk", "wkd", "wv", "wvd", "lam", "ident")

_cache = {}
DRAIN_BUDGET = 100   # 0 = no interleave (debug)


def build_nc():
    nc = bacc.Bacc("TRN2", target_bir_lowering=False, debug=False)

    x8_d = nc.declare_dram_parameter("x8", [128, DP, SC, 2, 512], F8, isOutput=False)
    xd8_d = nc.declare_dram_parameter("xd8", [128, DP, SC, 2, 512], F8, isOutput=False)
    w_names = ("wq", "wk", "wv")
    w_d = {n: nc.declare_dram_parameter(n, [128, DP, 2, DV], F8, isOutput=False)
           for n in w_names}
    wd_d = {n: nc.declare_dram_parameter(n + "d", [128, DP, 2, DV], F8, isOutput=False)
            for n in w_names}
    lam_d = nc.declare_dram_parameter("lam", [128, 1], F32, isOutput=False)
    ident_d = nc.declare_dram_parameter("ident", [128, 128], BF16, isOutput=False)
    out_d = nc.declare_dram_parameter("out", [SQ, DV], F32, isOutput=True)

    out = out_d.ap()
    DR = mybir.MatmulPerfMode.DoubleRow

    with tile.TileContext(nc) as tc, ExitStack() as ctx:
        singles = ctx.enter_context(tc.tile_pool(name="singles", bufs=1))
        x_pool = ctx.enter_context(tc.tile_pool(name="x", bufs=3))
        xd_pool = ctx.enter_context(tc.tile_pool(name="xd", bufs=3))
        t_pool = ctx.enter_context(tc.tile_pool(name="t", bufs=2))
        c_pool = ctx.enter_context(tc.tile_pool(name="c", bufs=2))
        o_pool = ctx.enter_context(tc.tile_pool(name="o", bufs=4))
        r_pool = ctx.enter_context(tc.tile_pool(name="r", bufs=2))

        # --- resident SBUF tensors -------------------------------------
        w_sb = {n: singles.tile([128, DP, 2, DV], F8, tag=f"w_{n}", name=f"w_{n}")
                for n in w_names}
        wd_sb = {n: singles.tile([128, DP, 2, DV], F8, tag=f"wd_{n}", name=f"wd_{n}")
                 for n in w_names}
        lam_sb = singles.tile([128, 1], F32, tag="lam")
        id_sb = singles.tile([128, 128], BF16, tag="ident")
        nc.sync.dma_start(out=lam_sb, in_=lam_d.ap())
        nc.sync.dma_start(out=id_sb, in_=ident_d.ap())

        nc.sync.dma_start(out=w_sb["wk"], in_=w_d["wk"].ap())

        kT = singles.tile([128, 2, S], BF16, tag="kT")        # [dh, head, sk]
        qT = singles.tile([128, 2, SQ], BF16, tag="qT")       # [dh, head, sq]
        v_sb = singles.tile([128, SKT, DV], BF16, tag="v")    # [s_row, s_tile, dv]
        et_all = singles.tile([128, 2, SKP + 2, 1024], BF16, tag="et")  # per-qc exp tiles
        # pairs 0,1 alternate buffers by qc parity so the next q-chunk's exp
        # need not wait for this chunk's combine of the same pair
        et_slot = lambda qc, p: p if p >= 2 else (SKP + p if qc % 2 else p)
        ones_sb = singles.tile([128, 1], BF16, tag="ones")    # = WSCALE for sum matmuls
        sums_sb = singles.tile([128, 8], F32, tag="sums_sb")
        nc.vector.memset(ones_sb, WSCALE)

        # --- psum pools: 4 + 1 + 1 + 2 = 8 banks -----------------------
        psum_s = ctx.enter_context(
            tc.tile_pool(name="psum_s", bufs=2, space=bass.MemorySpace.PSUM)
        )
        psum_aux = ctx.enter_context(
            tc.tile_pool(name="psum_aux", bufs=1, space=bass.MemorySpace.PSUM)
        )
        psum_pv = ctx.enter_context(
            tc.tile_pool(name="psum_pv", bufs=1, space=bass.MemorySpace.PSUM)
        )

        def wide_ps(name):
            return psum_s.tile([128, 1024], F32, tag="wide_ps", bufs=2, name=name)

        # x DMA: one transfer per tensor per chunk
        x_tiles = {}

        def load_x(sc):
            if sc in x_tiles:
                return
            xt = x_pool.tile([128, DP, 2, 512], F8, tag="xt", name=f"xt{sc}")
            nc.sync.dma_start(out=xt, in_=x8_d.ap()[:, :, sc, :, :])
            xdt = xd_pool.tile([128, DP, 2, 512], F8, tag="xdt", name=f"xdt{sc}")
            nc.sync.dma_start(out=xdt, in_=xd8_d.ap()[:, :, sc, :, :])
            x_tiles[sc] = (xt, xdt)

        load_x(0)
        for n in ("wv", "wq"):
            nc.sync.dma_start(out=w_sb[n], in_=w_d[n].ap())
        for n in ("wk", "wv", "wq"):
            nc.sync.dma_start(out=wd_sb[n], in_=wd_d[n].ap())

        # PE warm-up over the initial DMA wait
        jt = singles.tile([128, 512], BF16, tag="junk")
        nc.vector.memset(jt, 0.0)
        jps = wide_ps("jps")
        for w in range(40):
            nc.tensor.matmul(jps[:, 0:512], jt[:, 0:128], jt, start=True, stop=True)
        nc.vector.tensor_copy(jt, jps[:, 0:512])

        # --- projection emitters (psum borrowed from the wide ring) ----
        def proj_kq(wname, dst, sc, h):
            xt, xdt = x_tiles[sc]
            ps = wide_ps(f"ps{sc}{wname}{h}")
            gl = [(w_sb[wname], xt), (w_sb[wname], xdt), (wd_sb[wname], xt)]
            for g, (wt, xl) in enumerate(gl):
                for dp in range(DP):
                    nc.tensor.matmul(
                        ps[:, 0:512], wt[:, dp, :, h * HD:(h + 1) * HD],
                        xl[:, dp, :, :],
                        start=(g == 0 and dp == 0),
                        stop=(g == 2 and dp == DP - 1),
                        perf_mode=DR,
                    )
            nc.vector.tensor_copy(dst[:, h, sc * 512:(sc + 1) * 512], ps[:, 0:512])

        def proj_v(sc, i):
            xt, xdt = x_tiles[sc]
            ps = wide_ps(f"vps{sc}_{i}")
            gl = [(xt, w_sb["wv"]), (xdt, w_sb["wv"]), (xt, wd_sb["wv"])]
            for g, (xl, wt) in enumerate(gl):
                for dp in range(DP):
                    nc.tensor.matmul(
                        ps[:, 0:256], xl[:, dp, :, i * 128:(i + 1) * 128],
                        wt[:, dp, :, :],
                        start=(g == 0 and dp == 0),
                        stop=(g == 2 and dp == DP - 1),
                        perf_mode=DR,
                    )
            nc.vector.tensor_copy(v_sb[:, sc * 4 + i, :], ps[:, 0:256])

        # chunks 0-3: K and Q emitted sequentially (the attention stream
        # needs full qT + early kT); their V projections join the filler so
        # the exp stream starts ~20us earlier.
        filler = deque()
        emitted = set()
        for sc in range(QC):
            load_x(sc)
            for h in range(2):
                proj_kq("wk", kT, sc, h)
            for h in range(2):
                proj_kq("wq", qT, sc, h)
            if sc == 0:
                for i in range(4):
                    proj_v(sc, i)
        for sc in range(1, QC):
            for i in range(4):
                filler.append((1300, ("V", sc, i),
                               lambda sc=sc, i=i: proj_v(sc, i)))

        # prefetch + queue chunks 4-7 as attention-stream filler
        for sc in range(QC, SC):
            load_x(sc)
            for h in range(2):
                filler.append((2600, ("K", sc, h),
                               lambda sc=sc, h=h: proj_kq("wk", kT, sc, h)))
            for i in range(4):
                filler.append((1300, ("V", sc, i),
                               lambda sc=sc, i=i: proj_v(sc, i)))

        def drain(budget_ns):
            while filler and budget_ns > 0:
                ns, key, fn = filler.popleft()
                fn()
                emitted.add(key)
                budget_ns -= ns

        def drain_until(key):
            while filler and key not in emitted:
                ns, k, fn = filler.popleft()
                fn()
                emitted.add(k)

        # --- attention ---------------------------------------------------
        def emit_sums(sums_ps, qc, h, p, first, last):
            pm = et_slot(qc, p)
            for j in range(2):
                for sub in range(4):
                    nc.tensor.matmul(
                        sums_ps[:, 2 * sub + h:2 * sub + h + 1],
                        et_all[:, h, pm, j * 512 + sub * 128:j * 512 + (sub + 1) * 128],
                        ones_sb,
                        start=(first and j == 0 and sub == 0),
                        stop=(last and j == 1 and sub == 3),
                        skip_group_check=True,
                    )

        def queue_phase_b(qc, sums_ps):
            # sums -> (s1,s2) rows -> c = lam*s1/s2 (bf16, partition-bcast)
            nc.vector.tensor_copy(sums_sb, sums_ps)
            # c = lam*s1/s2 computed q-partitioned (partition-base-0 ops),
            # then bf16 column transposes assemble the [1,512] row.
            rcp2 = r_pool.tile([128, 4], F32, tag="rcp2", name=f"rcp2{qc}")
            ccol = r_pool.tile([128, 4], F32, tag="ccol", name=f"ccol{qc}")
            cbf = r_pool.tile([128, 4], BF16, tag="cbf", name=f"cbf{qc}")
            r1s = r_pool.tile([128, 4], F32, tag="r1s", name=f"r1s{qc}")
            for sub in range(4):
                nc.vector.reciprocal(rcp2[:, sub:sub + 1],
                                     sums_sb[:, 2 * sub + 1:2 * sub + 2])
                nc.vector.tensor_mul(ccol[:, sub:sub + 1],
                                     sums_sb[:, 2 * sub:2 * sub + 1],
                                     rcp2[:, sub:sub + 1])
                nc.vector.reciprocal(r1s[:, sub:sub + 1],
                                     sums_sb[:, 2 * sub:2 * sub + 1])
            nc.vector.tensor_scalar_mul(cbf, ccol, lam_sb)
            c_full = c_pool.tile([128, 1024], BF16, tag="cfull", name=f"cfull{qc}")

            def reduce_unit():
                trn_ps = psum_aux.tile([1, 512], BF16, tag="trn", name=f"trn{qc}")
                for sub in range(4):
                    nc.tensor.matmul(
                        trn_ps[:, sub * 128:(sub + 1) * 128],
                        cbf[:, sub:sub + 1],
                        id_sb, is_transpose=True, start=True, stop=True,
                        skip_group_check=True,
                    )
                cb = r_pool.tile([1, 512], BF16, tag="cb", name=f"cb{qc}")
                nc.vector.tensor_copy(cb, trn_ps)
                nc.gpsimd.partition_broadcast(c_full[:, 0:512], cb)
                nc.gpsimd.partition_broadcast(c_full[:, 512:1024], cb)
            filler.append((250, ("red", qc), reduce_unit))

            pv_all = psum_pv.tile([128, 4, DV], F32, tag="pv", name=f"pv{qc}")
            filler.append((300, ("bm", qc), lambda: nc.vector.memset(pv_all, 0.0)))

            def combine_pair(p):
                pm = et_slot(qc, p)
                def fn():
                    tmp = t_pool.tile([128, 1024], BF16, tag="tmp", name=f"tmp{qc}_{p}")
                    nc.vector.tensor_mul(tmp, et_all[:, 1, pm, :], c_full)
                    nc.vector.tensor_sub(et_all[:, 1, pm, :], et_all[:, 0, pm, :], tmp)
                return fn

            def pv_pair(p):
                pm = et_slot(qc, p)
                def fn():
                    for j in range(2):
                        skt = 2 * p + j
                        for i in range(4):
                            nc.tensor.matmul(
                                pv_all[:, i, :],
                                et_all[:, 1, pm, j * 512 + i * 128:j * 512 + (i + 1) * 128],
                                v_sb[:, skt, :],
                                start=False,
                                stop=(p == SKP - 1 and j == 1 and i == 3),
                                skip_group_check=True,
                            )
                return fn
            # combine (DVE) leads its PV (PE) by 2 units so the vector engine
            # has the data ready when the PE reaches the matmuls.
            for p in range(SKP + 2):
                if p < SKP:
                    filler.append((150, ("bc", qc, p), combine_pair(p)))
                if p >= 2:
                    filler.append((900, ("pv", qc, p - 2), pv_pair(p - 2)))

            def finish():
                for i in range(4):
                    idx = qc * 4 + i
                    ot = o_pool.tile([128, DV], F32, tag="ot", name=f"ot_{idx}")
                    nc.vector.tensor_scalar_mul(ot, pv_all[:, i, :], r1s[:, i:i + 1])
                    nc.sync.dma_start(out=out[idx * 128:(idx + 1) * 128, :], in_=ot)
            filler.append((200, ("bf", qc), finish))

        for qc in range(QC):
            sums_ps = psum_aux.tile([128, 8], F32, tag="sums", name=f"sums{qc}")
            pend = deque()
            slots = ([(h, p) for p in (0, 1) for h in (0, 1)]
                     + [(h, p) for p in range(2, SKP) for h in (0, 1)])
            for si, (h, p) in enumerate(slots):
                    # read-before-write guards: K proj for this pair's chunk;
                    # prev q-chunk's combine (h0 reads et[0,p]) / PV (h1
                    # overwrites the combined et[1,p]). Pairs 0,1 are
                    # double-buffered and need no guard.
                    if p >= 8:
                        drain_until(("K", p // 2, h))
                    if qc > 0 and p >= 2:
                        drain_until(("bc" if h == 0 else "pv", qc - 1, p))
                    sps = wide_ps(f"sps{qc}_{h}_{p}")
                    for j in range(2):
                        skt = 2 * p + j
                        nc.tensor.matmul(
                            sps[:, j * 512:(j + 1) * 512],
                            kT[:, h, skt * 128:(skt + 1) * 128],
                            qT[:, h, qc * 512:(qc + 1) * 512],
                            start=True, stop=True,
                        )
                    nc.scalar.activation(
                        out=et_all[:, h, et_slot(qc, p), :], in_=sps,
                        func=mybir.ActivationFunctionType.Exp,
                        scale=SCALE,
                    )
                    pend.append((h, p))
                    if len(pend) > 2:
                        hh, pp = pend.popleft()
                        emit_sums(sums_ps, qc, hh, pp,
                                  first=(hh == 0 and pp == 0), last=False)
                    drain(DRAIN_BUDGET)
            while pend:
                hh, pp = pend.popleft()
                emit_sums(sums_ps, qc, hh, pp, first=(hh == 0 and pp == 0),
                          last=(hh == 1 and pp == SKP - 1))
            queue_phase_b(qc, sums_ps)
            if DRAIN_BUDGET == 0:
                drain(float("inf"))
        drain(float("inf"))

    nc.compile()
    return nc


def _lam(lambda_q1, lambda_q2, lambda_k1, lambda_k2):
    lam_init = 0.8 - 0.6 * math.exp(-0.3 * DEPTH)
    l1 = math.exp(float(np.sum(lambda_q1.astype(np.float64) * lambda_k1.astype(np.float64))))
    l2 = math.exp(float(np.sum(lambda_q2.astype(np.float64) * lambda_k2.astype(np.float64))))
    return l1 + l2 + lam_init


def _pack_x(xT):
    """[D, S] f32 -> fp8 main + fp8 residual, packed [128, DP, SC, 2, 512]."""
    x8 = xT.astype(FP8NP)
    xd8 = (xT - x8.astype(np.float32)).astype(FP8NP)
    def pack(a):
        return np.ascontiguousarray(
            a.reshape(DP, 2, 128, SC, 512).transpose(2, 0, 3, 1, 4))
    return pack(x8), pack(xd8)


def _pack_w(W):
    """[D, 256] f32 -> x64-scaled fp8 main + residual, packed [128, DP, 2, 256]."""
    Ws = W * WSCALE
    w8 = Ws.astype(FP8NP)
    wd8 = (Ws - w8.astype(np.float32)).astype(FP8NP)
    def pack(a):
        return np.ascontiguousarray(
            a.reshape(DP, 2, 128, DV).transpose(2, 0, 1, 3))
    return pack(w8), pack(wd8)


def kernel(x, WQ, WK, WV, lambda_q1, lambda_q2, lambda_k1, lambda_k2):
    if "nc" not in _cache:
        _cache["nc"] = build_nc()
    nc = _cache["nc"]

    lam = np.full((128, 1), _lam(lambda_q1, lambda_q2, lambda_k1, lambda_k2), np.float32)
    ident = np.eye(128).astype(ml_dtypes.bfloat16)
    wq8, wq8d = _pack_w(np.asarray(WQ, np.float32))
    wk8, wk8d = _pack_w(np.asarray(WK, np.float32))
    wv8, wv8d = _pack_w(np.asarray(WV, np.float32))

    in_maps = []
    for c in range(N_CORES):
        b, qs = c // 2, (c % 2) * SQ
        xb = x[b] if qs == 0 else np.concatenate([x[b, qs:], x[b, :qs]], axis=0)
        xT = np.ascontiguousarray(xb.T, dtype=np.float32)
        x8, xd8 = _pack_x(xT)
        in_maps.append({"x8": x8, "xd8": xd8,
                        "wq": wq8, "wqd": wq8d,
                        "wk": wk8, "wkd": wk8d,
                        "wv": wv8, "wvd": wv8d,
                        "lam": lam, "ident": ident})

    kres = None
    for attempt in range(3):
        try:
            kres = run_bass_kernel_spmd(nc, in_maps, list(range(N_CORES)))
            break
        except (ModuleNotFoundError, ImportError):
            os.environ["BASS_NEVER_TRACE"] = "1"
        except Exception:
            if attempt == 2:
                raise
            time.sleep(5)
    if kres is None:
        kres = run_bass_kernel_spmd(nc, in_maps, list(range(N_CORES)))
    _cache["last_results"] = kres
    res = kres.results

    out = np.empty((B, S, DV), np.float32)
    for c in range(N_CORES):
        b, qs = c // 2, (c % 2) * SQ
        out[b, qs:qs + SQ] = res[c]["out"]
    return out
